# revision 51
# baseline (speedup 1.0000x reference)
"""Trainium2 Bass kernel for nn_DeepONetCfCDecoder.

Strategy: queries are sorted by their searchsorted time index on host and
split into 8 contiguous chunks (one per core). Each core then only needs a
small window of W_T timesteps of h_states, so the per-query sparse gather
becomes a dense block-local cross-attention: scores are computed key-major
against the whole window and masked with an exact (timestep == query-idx)
comparison applied as a {0,1} multiply on exp(scores) (no max subtraction
needed -- scores are O(1) here and masking is exact).

Everything on device is feature-major [feat partitions x 512 queries free],
so all linears are plain PE matmuls (float32r, full-rate) with weights as
stationary operands and no activation transposes anywhere. LayerNorms
reduce across partitions with ones-vector matmuls; row->tile broadcasts are
rank-1 PE outer products; rsqrt for the two layernorms uses the integer
bit-trick + Newton steps on the vector engine so the scalar engine only
ever loads the Sin and Exp activation tables.

Host-side work is limited to index computation (searchsorted/argsort),
window slicing/layout, and parameter folding (bias folds, LN-gain folds,
tok_w composed into the k/v projections, and the degenerate size-1-axis LN
branch which collapses to a constant).
"""

import os
import sys

import ml_dtypes
import numpy as np

for _p in ("/opt/trn_rl_repo", "/root/.axon_site/_ro/trn_rl_repo"):
    if os.path.isdir(_p) and _p not in sys.path:
        sys.path.append(_p)

import concourse.bacc as bacc
import concourse.bass as bass
import concourse.mybir as mybir
import concourse.tile as tile

N, K, T = 4096, 48, 200
H, RANK, FH = 256, 128, 16
KP = 64          # sensors padded per timestep (48 -> 64) so key-blocks align
NCORE = 8
NC = N // NCORE  # queries per core
FP = mybir.dt.float32
FR = mybir.dt.float32r
I32 = mybir.dt.int32
BF = mybir.dt.bfloat16
AF = mybir.ActivationFunctionType
ALU = mybir.AluOpType
EPS = 1e-5
RSQRT_MAGIC = 0x5F3759DF

# cpack column map
C_ONES = 0      # 2 cols of 1.0
C_SPX, C_SPY = 2, 3
C_PENRC = 4
C_CV = 5        # 2 cols
C_BCB2 = 7      # 2
C_BEFF = 9      # 2
C_NBEFF = 11    # 2
C_QB = 13       # 2
C_TOB = 15      # 3
C_B1 = 18       # 2
C_NB1 = 20      # 2
C_BPB = 22      # 3
C_TS = 25       # 2 (duplicated)
C_TB = 27       # 2 (duplicated)
C_KEYT = 29     # NKB cols


def _tiled(w, fout):
    """[fin, fout] -> [128, nb*fout] with fin = nb*128 partition-tiled."""
    fin = w.shape[0]
    nb = fin // 128
    return np.ascontiguousarray(
        w.reshape(nb, 128, fout).transpose(1, 0, 2).reshape(128, nb * fout)
    ).astype(np.float32)


def _silu(x):
    return x / (1.0 + np.exp(-x))


def _host_prep(xy, t_q, c, h_states, sensor_time, sensor_pos, params):
    p = {k: np.asarray(v, np.float32) for k, v in params.items()}
    xy = np.asarray(xy, np.float32)
    t_q = np.asarray(t_q, np.float32)
    c = np.asarray(c).astype(np.int64)
    h_states = np.asarray(h_states, np.float32)
    st = np.asarray(sensor_time, np.float32)
    sp = np.asarray(sensor_pos, np.float32)

    idx = np.clip(np.searchsorted(st, t_q, side='right') - 1, 0, T - 1)
    order = np.argsort(idx, kind='stable')
    inv_order = np.argsort(order)
    idx_s = idx[order]
    xy_s = xy[order]
    dt_s = np.maximum(t_q[order] - st[idx_s], 0.0)
    c_s = c[order]

    starts, spans = [], []
    for m in range(NCORE):
        lo, hi = int(idx_s[m * NC]), int(idx_s[(m + 1) * NC - 1])
        starts.append(lo)
        spans.append(hi - lo + 1)
    W_T = max(spans)
    W_T += (-W_T) % 8  # WK divisible by 512
    W_T = min(W_T, ((T + 7) // 8) * 8)
    assert W_T >= max(spans)
    starts = [max(0, min(s, T - W_T)) for s in starts]
    WK = W_T * KP
    NKB = WK // 128

    # ---- parameter folds ----
    tin = p['trunk_in_w']
    W68 = np.concatenate([tin[:64],
                          p['time_proj_w'][0:1] @ tin[64:96],
                          p['comp_emb'] @ tin[96:104]], axis=0)
    b_eff = p['trunk_in_b'] + p['time_proj_b'] @ tin[64:96]
    qw_fold = (p['bn_g'][:, None] * p['q_w']) / np.sqrt(H)
    qb_fold = (p['bn_b'] @ p['q_w'] + p['q_b']) / np.sqrt(H)
    cv = p['tok_b'] @ p['v_w'] + p['v_b']
    kw_eff = p['tok_w'] @ p['k_w']      # tokens fold: k = (h@tok_w)@k_w
    vw_eff = p['tok_w'] @ p['v_w']
    w1_fold = p['bc_ln_g'][:, None] * p['bc_w1']
    b1_fold = p['bc_ln_b'] @ p['bc_w1'] + p['bc_b1']
    # rel-pos MLP: LN over a size-1 axis is constant => whole branch is const
    h1 = _silu(p['rp_ln_b'][0] * p['rp_w1'][0] + p['rp_b1'])
    rc = float(h1 @ p['rp_w2'][:, 0] + p['rp_b2'][0])
    decay = float(np.exp(p['log_locality_decay'][0]))
    temp = float(np.exp(p['log_fusion_temperature'][0]))
    ts_vec = (temp * p['component_scale']).astype(np.float32)
    tb_vec = p['component_bias'].astype(np.float32)

    weights = {
        'w68': np.ascontiguousarray(W68, np.float32),          # [68,256]
        'kwe': _tiled(kw_eff, 256).astype(ml_dtypes.bfloat16),
        'vwe': _tiled(vw_eff, 256).astype(ml_dtypes.bfloat16),
        'qw': _tiled(qw_fold, 256),
        'tow': _tiled(p['trunk_out_w'], 384),
        'w1': _tiled(w1_fold, 256),
        'w2': _tiled(p['bc_w2'], 256),
        'bpw': _tiled(p['branch_proj_w'], 384),
    }

    cpack = np.zeros((128, C_KEYT + NKB), np.float32)
    cpack[:, C_ONES] = 1.0
    cpack[:, C_ONES + 1] = 1.0
    spp = np.zeros((KP, 2), np.float32)
    spp[:48] = sp
    cpack[:, C_SPX] = np.tile(spp[:, 0], 2)
    cpack[:, C_SPY] = np.tile(spp[:, 1], 2)
    pen = np.zeros(128, np.float32)
    pen[48:64] = -30.0
    pen[112:128] = -30.0
    cpack[:, C_PENRC] = pen + rc

    def put(col, vec):
        v = np.asarray(vec, np.float32).reshape(-1)
        nb = v.size // 128
        for i in range(nb):
            cpack[:, col + i] = v[i * 128:(i + 1) * 128]

    put(C_CV, cv)
    put(C_BCB2, p['bc_b2'])
    put(C_BEFF, b_eff)
    put(C_NBEFF, -b_eff)
    put(C_QB, qb_fold)
    put(C_TOB, p['trunk_out_b'])
    put(C_B1, b1_fold)
    put(C_NB1, -b1_fold)
    put(C_BPB, p['branch_proj_b'])
    kt = (2 * np.arange(NKB)[None, :] + (np.arange(128) >= 64)[:, None])
    cpack[:, C_KEYT:C_KEYT + NKB] = kt.astype(np.float32)
    cpfr = np.zeros((128, 6), np.float32)
    cpfr[:, 0:2] = 1.0
    cpfr[0:3, 2] = ts_vec
    cpfr[0:3, 3] = ts_vec
    cpfr[0:3, 4] = tb_vec
    cpfr[0:3, 5] = tb_vec

    kh = np.arange(1, FH + 1, dtype=np.float64)
    cores = []
    for m in range(NCORE):
        s = slice(m * NC, (m + 1) * NC)
        start = starts[m]
        hw = np.zeros((W_T, KP, 384), np.float32)
        hw[:, :48] = h_states[start:start + W_T]
        hT = _tiled(hw.reshape(WK, 384).T.copy().reshape(384, WK),
                    WK).astype(ml_dtypes.bfloat16)
        tl = (idx_s[s] - start).astype(np.float32)
        oh = np.zeros((3, NC), np.float32)
        oh[c_s[s], np.arange(NC)] = 1.0
        aux = np.concatenate([dt_s[s][None], oh, tl[None]], 0)  # [5,NC]
        xs = xy_s[s].astype(np.float64)
        ang = 2 * np.pi * xs[:, :, None] * kh                   # [NC,2,FH]
        A = np.concatenate([ang, ang + np.pi / 2], axis=2).reshape(NC, 64).T
        A = np.mod(A, 2 * np.pi)
        A = np.where(A > np.pi, A - 2 * np.pi, A)
        A = np.clip(A, -np.pi, np.pi).astype(np.float32)
        core = dict(hT=hT, aux=np.ascontiguousarray(aux),
                    a=np.ascontiguousarray(A),
                    xyt=np.ascontiguousarray(xy_s[s].T), cpack=cpack,
                    cpfr=cpfr, onesr=np.ones((1, 128), np.float32),
                    ones2b=np.ones((128, 2), ml_dtypes.bfloat16))
        core.update(weights)
        cores.append(core)
    return cores, W_T, decay, inv_order


_PROG_CACHE = {}


def _build(W_T, decay):
    WK = W_T * KP
    NKB = WK // 128
    WKC = WK // 512
    NCP = C_KEYT + NKB

    nc = bacc.Bacc(None, target_bir_lowering=False, debug=False)
    d_hT = nc.dram_tensor("hT", [128, 3 * WK], BF, kind="ExternalInput")
    d_a = nc.dram_tensor("a", [64, NC], FP, kind="ExternalInput")
    d_aux = nc.dram_tensor("aux", [5, NC], FP, kind="ExternalInput")
    d_xyt = nc.dram_tensor("xyt", [2, NC], FP, kind="ExternalInput")
    d_cpack = nc.dram_tensor("cpack", [128, NCP], FP, kind="ExternalInput")
    d_cpfr = nc.dram_tensor("cpfr", [128, 6], FR, kind="ExternalInput")
    d_onesr = nc.dram_tensor("onesr", [1, 128], FR, kind="ExternalInput")
    d_ones2b = nc.dram_tensor("ones2b", [128, 2], BF, kind="ExternalInput")
    d_w68 = nc.dram_tensor("w68", [68, 256], FR, kind="ExternalInput")
    WSPEC = [('kwe', 3, 256, BF), ('vwe', 3, 256, BF), ('qw', 2, 256, FR),
             ('tow', 2, 384, FR), ('w1', 2, 256, FR), ('w2', 2, 256, FR),
             ('bpw', 2, 384, FR)]
    dw = {}
    for name, nb, fo, dt_ in WSPEC:
        dw[name] = nc.dram_tensor(name, [128, nb * fo], dt_,
                                  kind="ExternalInput")
    d_out = nc.dram_tensor("out", [1, NC], FP, kind="ExternalOutput")

    def fr(ap):
        return ap if ap.dtype == FR else ap.bitcast(FR)

    with tile.TileContext(nc) as tc:
        PSUM = bass.MemorySpace.PSUM
        with (
            nc.allow_low_precision(reason="float32r-rounded matmul operands"),
            tc.tile_pool(name="big", bufs=1) as big,
            tc.tile_pool(name="w", bufs=1) as wpool,
            tc.tile_pool(name="act", bufs=1) as act,
            tc.tile_pool(name="epool", bufs=3) as epool,
            tc.tile_pool(name="tmp", bufs=2) as tmp,
            tc.tile_pool(name="rowp", bufs=1) as rowp,
            tc.tile_pool(name="rows", bufs=4) as rows,
            tc.tile_pool(name="ps", bufs=4, space=PSUM) as ps,
            tc.tile_pool(name="psacc", bufs=1, space=PSUM) as psacc,
            tc.tile_pool(name="psrow", bufs=1, space=PSUM) as psrow,
        ):
            dmae = [nc.sync, nc.scalar, nc.gpsimd]

            def dma(i, dst, src):
                dmae[i % 3].dma_start(dst, src)

            # ---------- loads: small tensors first, then weights, then hT
            # streamed per (fin-block, key-chunk) so compute starts early ----
            a_sb = act.tile([64, NC], FP)
            dma(2, a_sb[:], d_a[:])
            cp = wpool.tile([128, NCP], FP)
            dma(0, cp[:], d_cpack[:])
            cpf = wpool.tile([128, 6], FR)
            dma(1, cpf[:], d_cpfr[:])
            w68 = wpool.tile([68, 256], FR)
            dma(1, w68[:], d_w68[:])
            xr = rowp.tile([1, NC], FR, tag="xr")
            dma(0, xr[:], d_xyt[0:1, :].bitcast(FR))
            yr = rowp.tile([1, NC], FR, tag="yr")
            dma(1, yr[:], d_xyt[1:2, :].bitcast(FR))
            oh_sb = rowp.tile([3, NC], FR)
            dma(2, oh_sb[:], d_aux[1:4, :].bitcast(FR))
            ohr = []
            for j in range(3):
                t = rowp.tile([1, NC], FP, name=f"ohr_{j}", tag=f"ohr_{j}")
                dma(j, t[:], d_aux[1 + j:2 + j, :])
                ohr.append(t)
            tlr = rowp.tile([1, NC], FR)
            dma(0, tlr[:], d_aux[4:5, :].bitcast(FR))
            ones_r = rowp.tile([1, 128], FR)
            dma(2, ones_r[:], d_onesr[:])
            ones2b = wpool.tile([128, 2], BF, name="ones2b", tag="ones2b")
            dma(0, ones2b[:], d_ones2b[:])
            x68 = act.tile([68, NC], FR)
            dma(1, x68[64:68, :], d_aux[0:4, :].bitcast(FR))
            wt = {}
            worder = ['tow', 'qw', 'w1', 'w2', 'bpw', 'kwe', 'vwe']
            wspec_d = {n: (nb, fo, dt_) for n, nb, fo, dt_ in WSPEC}
            for i, name in enumerate(worder):
                nb, fo, dt_ = wspec_d[name]
                wt[name] = wpool.tile([128, nb, fo], dt_, name=f"w_{name}",
                                      tag=f"w_{name}")
                dma(2 * i, wt[name][:],
                    dw[name][:].rearrange("p (b f) -> p b f", b=nb))
            hT = big.tile([128, 3, WK], BF)
            for ch in range(WKC):
                for b in range(3):
                    cs = slice(ch * 512, (ch + 1) * 512)
                    dma(2 * (ch * 3 + b), hT[:, b, cs],
                        d_hT[:, b * WK + ch * 512:b * WK + (ch + 1) * 512])
            ones2 = cpf[:, 0:2]

            def mm(out, lhsT, rhs, **kw):
                nc.tensor.matmul(out, fr(lhsT), fr(rhs), **kw)

            def outer(row_ap, dst_sb=None):
                """[1,w] row -> [128,w] broadcast; returns PSUM tile or
                evicts into dst_sb (via ACT copy) if given."""
                w_ = row_ap.shape[-1]
                o = ps.tile([128, w_], FP, name="o", tag="mm")
                mm(o[:], ones_r[:], row_ap, start=True, stop=True)
                if dst_sb is None:
                    return o
                nc.scalar.copy(dst_sb, o[:])
                return None

            def colsum(x2, nfi):
                """sum over feature partitions of [128,nfi,NC] -> psum
                [2,NC] tile, row 0 is the sum."""
                s = psrow.tile([2, NC], FP, name="s", tag="row")
                for fi in range(nfi):
                    if x2.dtype == BF:
                        nc.tensor.matmul(s[:], ones2b[:], x2[:, fi, :],
                                         start=(fi == 0),
                                         stop=(fi == nfi - 1))
                    else:
                        mm(s[:], ones2, x2[:, fi, :],
                           start=(fi == 0), stop=(fi == nfi - 1))
                return s

            def rsqrt_row(dst, src, use_ars):
                """dst = 1/sqrt(src), src > 0. ACT table op when the load
                is amortized; int bit-trick + Newton on DVE otherwise."""
                if use_ars:
                    nc.scalar.activation(dst, src, AF.Abs_reciprocal_sqrt)
                    return
                yi = rows.tile([1, NC], FP, name="yi", tag="r")
                nc.vector.tensor_scalar(yi[:].bitcast(I32),
                                        src.bitcast(I32), 1, None,
                                        op0=ALU.arith_shift_right)
                nc.vector.tensor_scalar(yi[:].bitcast(I32),
                                        yi[:].bitcast(I32), -1, RSQRT_MAGIC,
                                        op0=ALU.mult, op1=ALU.add)
                t = rows.tile([1, NC], FP, name="t", tag="r")
                for _ in range(2):
                    nc.vector.tensor_mul(t[:], yi[:], yi[:])
                    nc.vector.tensor_mul(t[:], t[:], src)
                    nc.vector.tensor_scalar(t[:], t[:], -0.5, 1.5,
                                            op0=ALU.mult, op1=ALU.add)
                    nc.vector.tensor_mul(yi[:], yi[:], t[:])
                nc.vector.tensor_copy(dst, yi[:])

            # ---------- trunk features ----------
            nc.scalar.activation(x68[0:64, :], a_sb[:], AF.Sin)

            def silu_evict(dst, psin, bcol, nbcol):
                """dst = silu(psin + bias); bias col from cpack."""
                nc.scalar.activation(dst, psin, AF.Silu,
                                     bias=cp[:, bcol:bcol + 1])

            tf = act.tile([128, 2, NC], FR)
            for fo in range(2):
                p0 = ps.tile([128, NC], FP, name="p0", tag="mm")
                mm(p0[:], w68[:, fo * 128:(fo + 1) * 128], x68[:],
                   start=True, stop=True)
                silu_evict(tf[:, fo, :].bitcast(FR), p0[:], C_BEFF + fo,
                           C_NBEFF + fo)

            tb = act.tile([128, 3, NC], FP)
            for fo in range(3):
                p0 = ps.tile([128, NC], FP, name="p0", tag="mm")
                for fi in range(2):
                    mm(p0[:], wt['tow'][:, fi, fo * 128:(fo + 1) * 128],
                       tf[:, fi, :], start=(fi == 0), stop=(fi == 1))
                nc.scalar.activation(tb[:, fo, :], p0[:], AF.Identity,
                                     bias=cp[:, C_TOB + fo:C_TOB + fo + 1])

            # ---------- bn layernorm on tf -> q ----------
            def ln_rows(x2, use_ars=True):
                """x2: [128,2,NC] -> (a_row, c_row) [1,NC] SBUF rows with
                a = rsqrt(var+eps), c = -mean*a."""
                s1p = colsum(x2, 2)
                sq = tmp.tile([128, 2, NC], FP, name="sq", tag="sq")
                for fi in range(2):
                    nc.scalar.activation(sq[:, fi, :].bitcast(FR),
                                         x2[:, fi, :], AF.Square)
                s2p = colsum(sq, 2)
                mr = rows.tile([1, NC], FP, name="mr", tag="r")
                vr = rows.tile([1, NC], FP, name="vr", tag="r")
                ar = rows.tile([1, NC], FP, name="ar", tag="r")
                cr = rows.tile([1, NC], FP, name="cr", tag="r")
                nc.vector.tensor_scalar_mul(mr[:], s1p[0:1, :], 1.0 / H)
                nc.vector.tensor_mul(vr[:], mr[:], mr[:])
                nc.vector.scalar_tensor_tensor(vr[:], s2p[0:1, :], 1.0 / H,
                                               vr[:], op0=ALU.mult,
                                               op1=ALU.subtract)
                nc.vector.tensor_scalar_add(vr[:], vr[:], EPS)
                rsqrt_row(ar[:].bitcast(FR), vr[:], use_ars)
                nc.vector.scalar_tensor_tensor(cr[:].bitcast(FR), mr[:], -1.0,
                                               ar[:], op0=ALU.mult,
                                               op1=ALU.mult)
                return ar, cr

            ar, cr = ln_rows(tf)
            ab = outer(ar[:])
            cb = outer(cr[:])
            tfn = act.tile([128, 2, NC], FR)
            for fi in range(2):
                nc.vector.tensor_mul(tfn[:, fi, :].bitcast(FR),
                                     tf[:, fi, :], ab[:])
                nc.vector.tensor_add(tfn[:, fi, :].bitcast(FR),
                                     tfn[:, fi, :], cb[:])
            qfm = act.tile([128, 2, NC], FP)
            for fo in range(2):
                p0 = ps.tile([128, NC], FP, name="p0", tag="mm")
                for fi in range(2):
                    mm(p0[:], wt['qw'][:, fi, fo * 128:(fo + 1) * 128],
                       tfn[:, fi, :], start=(fi == 0), stop=(fi == 1))
                nc.scalar.activation(qfm[:, fo, :].bitcast(FR), p0[:],
                                     AF.Identity,
                                     bias=cp[:, C_QB + fo:C_QB + fo + 1])

            # ---------- expR: exp of wrapped-distance bias tile ----------
            expr = act.tile([128, NC], BF)
            d2 = tmp.tile([128, NC], FP, name="d2", tag="d2", bufs=1)
            for i, (crow, ccol) in enumerate(((xr, C_SPX), (yr, C_SPY))):
                xbp = outer(crow[:])
                rel = tmp.tile([128, NC], FP, name="rel", tag="rel", bufs=1)
                nc.vector.tensor_scalar(rel[:], xbp[:], cp[:, ccol:ccol + 1],
                                        None, op0=ALU.subtract)
                ax = tmp.tile([128, NC], FP, name="ax", tag="ax", bufs=1)
                nc.scalar.activation(ax[:], rel[:], AF.Abs)
                u = tmp.tile([128, NC], FP, name="u", tag="relu_", bufs=1)
                nc.vector.tensor_scalar(u[:], ax[:], -1.0, 1.0,
                                        op0=ALU.mult, op1=ALU.add)
                nc.vector.tensor_tensor(u[:], ax[:], u[:], op=ALU.min)
                if i == 0:
                    nc.vector.tensor_mul(d2[:], u[:], u[:])
                else:
                    nc.vector.tensor_mul(u[:], u[:], u[:])
                    nc.vector.tensor_add(d2[:], d2[:], u[:])
            # r = d2 * rsqrt(d2); ARS(0)=inf is clamped so r(0) = 0
            yt = tmp.tile([128, NC], FP, name="yt", tag="yt", bufs=1)
            nc.scalar.activation(yt[:], d2[:], AF.Abs_reciprocal_sqrt)
            nc.vector.tensor_scalar(yt[:], yt[:], 1e30, None, op0=ALU.min)
            nc.vector.tensor_mul(yt[:], yt[:], d2[:])
            nc.vector.tensor_scalar(yt[:], yt[:], -decay,
                                    cp[:, C_PENRC:C_PENRC + 1],
                                    op0=ALU.mult, op1=ALU.add)
            nc.scalar.activation(expr[:], yt[:], AF.Exp)

            tlb = act.tile([128, NC], BF)
            outer(tlr[:], tlb[:])

            # ---------- k over the key window (tokens folded into kwe) -----
            kfm = big.tile([128, 2, WK], FR)
            for ch in range(WKC):
                cs = slice(ch * 512, (ch + 1) * 512)
                for fo in range(2):
                    p0 = ps.tile([128, 512], FP, name="p0", tag="mm")
                    for fi in range(3):
                        nc.tensor.matmul(
                            p0[:], wt['kwe'][:, fi, fo * 128:(fo + 1) * 128],
                            hT[:, fi, cs], start=(fi == 0), stop=(fi == 2))
                    nc.scalar.copy(kfm[:, fo, cs], p0[:])
            vkm = big.tile([128, NKB, 256], BF)
            for kb in range(NKB):
                ks = slice(kb * 128, (kb + 1) * 128)
                p0 = ps.tile([128, 256], FP, name="p0", tag="mm")
                for fi in range(3):
                    nc.tensor.matmul(p0[:], hT[:, fi, ks], wt['vwe'][:, fi, :],
                                     start=(fi == 0), stop=(fi == 2))
                nc.vector.tensor_copy(vkm[:, kb, :], p0[:])

            # ---------- attention ----------
            ctx0 = psacc.tile([128, NC], FP, tag="ctx0")
            ctx1 = psacc.tile([128, NC], FP, tag="ctx1")
            ctxs = [ctx0, ctx1]
            den = psacc.tile([2, NC], FP, tag="den")
            for kb in range(NKB):
                ks = slice(kb * 128, (kb + 1) * 128)
                sp_ = ps.tile([128, NC], FP, name="sp_", tag="mm")
                for hb in range(2):
                    mm(sp_[:], kfm[:, hb, ks], qfm[:, hb, :],
                       start=(hb == 0), stop=(hb == 1))
                e1 = epool.tile([128, NC], BF, name="e1", tag="e1")
                nc.scalar.activation(e1[:], sp_[:], AF.Exp)
                mk = epool.tile([128, NC], BF, name="mk", tag="mk")
                kc = C_KEYT + kb
                nc.vector.scalar_tensor_tensor(
                    mk[:], tlb[:], cp[:, kc:kc + 1], expr[:],
                    op0=ALU.is_equal, op1=ALU.mult)
                E = epool.tile([128, NC], BF, name="E", tag="E")
                nc.vector.tensor_mul(E[:], e1[:], mk[:])
                for hb in range(2):
                    nc.tensor.matmul(
                        ctxs[hb][:], vkm[:, kb, hb * 128:(hb + 1) * 128], E[:],
                        start=(kb == 0), stop=(kb == NKB - 1),
                        skip_group_check=True)
                nc.tensor.matmul(den[:], ones2b[:], E[:], start=(kb == 0),
                                 stop=(kb == NKB - 1), skip_group_check=True)

            # ---------- normalize + bc block ----------
            # ctx = u * invd + cv directly (cv is per-partition, no den
            # broadcast needed; layernorm stats are computed on ctx itself)
            invd = rows.tile([1, NC], FP, name="invd", tag="r")
            nc.vector.reciprocal(invd[:].bitcast(FR), den[0:1, :])
            db = act.tile([128, NC], FP)
            outer(invd[:], db[:])

            up = act.tile([128, 2, NC], FR)
            for hb in range(2):
                t0 = tmp.tile([128, NC], FP, name="t0", tag="res")
                nc.vector.tensor_mul(t0[:], ctxs[hb][:], db[:])
                nc.vector.tensor_scalar_add(
                    up[:, hb, :].bitcast(FR), t0[:],
                    cp[:, C_CV + hb:C_CV + hb + 1])

            a2r, c2r = ln_rows(up)
            a2b = outer(a2r[:])
            c2b = outer(c2r[:])
            un = act.tile([128, 2, NC], FR)
            for fi in range(2):
                nc.vector.tensor_mul(un[:, fi, :].bitcast(FR),
                                     up[:, fi, :], a2b[:])
                nc.vector.tensor_add(un[:, fi, :].bitcast(FR),
                                     un[:, fi, :], c2b[:])
            h2 = act.tile([128, 2, NC], FR)
            for fo in range(2):
                p0 = ps.tile([128, NC], FP, name="p0", tag="mm")
                for fi in range(2):
                    mm(p0[:], wt['w1'][:, fi, fo * 128:(fo + 1) * 128],
                       un[:, fi, :], start=(fi == 0), stop=(fi == 1))
                silu_evict(h2[:, fo, :].bitcast(FR), p0[:], C_B1 + fo,
                           C_NB1 + fo)

            bp_in = act.tile([128, 2, NC], FR)
            for fo in range(2):
                p0 = ps.tile([128, NC], FP, name="p0", tag="mm")
                for fi in range(2):
                    mm(p0[:], wt['w2'][:, fi, fo * 128:(fo + 1) * 128],
                       h2[:, fi, :], start=(fi == 0), stop=(fi == 1))
                nc.vector.scalar_tensor_tensor(
                    bp_in[:, fo, :].bitcast(FR), p0[:],
                    cp[:, C_BCB2 + fo:C_BCB2 + fo + 1],
                    up[:, fo, :], op0=ALU.add, op1=ALU.add)

            # ---------- branch basis, fuse, output ----------
            pre = rows.tile([1, NC], FP, name="pre", tag="r")
            t2 = rows.tile([1, NC], FP, name="t2", tag="r")
            for fo in range(3):
                p0 = ps.tile([128, NC], FP, name="p0", tag="mm")
                for fi in range(2):
                    mm(p0[:], wt['bpw'][:, fi, fo * 128:(fo + 1) * 128],
                       bp_in[:, fi, :], start=(fi == 0), stop=(fi == 1))
                prod = tmp.tile([128, NC], FR, name="prod", tag="prod")
                nc.vector.scalar_tensor_tensor(
                    prod[:], p0[:], cp[:, C_BPB + fo:C_BPB + fo + 1],
                    tb[:, fo, :], op0=ALU.add, op1=ALU.mult)
                s3t = psrow.tile([2, NC], FP, name=f"s3_{fo}", tag="row")
                mm(s3t[:], ones2, prod[:], start=True, stop=True)
                if fo == 0:
                    nc.vector.tensor_mul(pre[:], s3t[0:1, :], ohr[0][:])
                else:
                    nc.vector.tensor_mul(t2[:], s3t[0:1, :], ohr[fo][:])
                    nc.vector.tensor_add(pre[:], pre[:], t2[:])
            tsr = psrow.tile([2, NC], FP, name="tsr", tag="row")
            tbr = psrow.tile([2, NC], FP, name="tbr", tag="row")
            mm(tsr[:], cpf[0:3, 2:4], oh_sb[:], start=True, stop=True)
            mm(tbr[:], cpf[0:3, 4:6], oh_sb[:], start=True, stop=True)
            outr = rows.tile([1, NC], FP, name="outr", tag="r")
            nc.vector.tensor_mul(outr[:], pre[:], tsr[0:1, :])
            nc.vector.tensor_add(outr[:], outr[:], tbr[0:1, :])
            nc.sync.dma_start(d_out[:], outr[:])

    nc.compile()
    return nc


def _get_prog(W_T, decay):
    key = (W_T, round(float(decay), 8))
    if key not in _PROG_CACHE:
        _PROG_CACHE[key] = _build(W_T, decay)
    return _PROG_CACHE[key]


def _run(xy, t_q, c, h_states, sensor_time, sensor_pos, params, **runkw):
    cores, W_T, decay, inv_order = _host_prep(
        xy, t_q, c, h_states, sensor_time, sensor_pos, params)
    nc = _get_prog(W_T, decay)
    from concourse.bass_utils import run_bass_kernel_spmd
    br = run_bass_kernel_spmd(nc, cores, list(range(NCORE)), **runkw)
    out = np.concatenate([np.asarray(r["out"]).reshape(NC)
                          for r in br.results])
    return out[inv_order][:, None].astype(np.float32), br


def kernel(xy, t_q, c, h_states, sensor_time, sensor_pos, params):
    return _run(xy, t_q, c, h_states, sensor_time, sensor_pos, params)[0]
